# revision 1
# baseline (speedup 1.0000x reference)
"""GCN (2-layer, GCNConv + log_softmax) on 8 Trainium2 NeuronCores.

Strategy (1D node partition, per sharding hint):
  - Nodes padded to N_PAD = 392*128 and sharded contiguously: 49 blocks of 128
    dst-nodes per core.
  - CPU preprocessing: add self-loops, compute symmetric norm, sort edges by
    dst, pack per (core, block) into fixed-size edge tiles of 128 (padded with
    norm=0 edges so all cores run an identical instruction stream).
  - On device per core:
      GEMM1: h = x_shard @ W1 (PE, bf16 operands, fp32 accum)
      AllGather h -> full h table in local HBM
      Agg1 per dst block: indirect-DMA gather h[src] for all edge tiles of the
        block, build scaled selector S[e,dst] = (seg[e]==dst)*norm[e] on DVE,
        segment-sum via PE matmul accumulation into PSUM [hid, dst];
        relu(agg+b1) on ACT; fused GEMM2 -> h2 block; store to h2 shard.
      AllGather h2 -> full h2 table
      Agg2 per dst block: gather h2[src], same selector, accumulate [dst, cls];
        +b2, log_softmax on DVE/ACT; store output shard.
  - Host concatenates the 8 output shards and strips padding.
"""

import math

import numpy as np
import ml_dtypes

P = 128
NCORES = 8

# Full-problem constants (hardcoded per harness contract).
N_NODES = 50000
N_EDGES = 800000
F_IN = 512
HIDDEN = 128
N_CLASSES = 40

# Runtime-tunable knobs (test.py may override before calling kernel()).
TRACE = False
TRACE_KWARGS = {}
H_DTYPE = "bfloat16"    # dtype of the h (layer-1 projected) gather table
H2_DTYPE = "float32"    # dtype of the h2 (layer-2 projected) gather table
X_DTYPE = "bfloat16"    # GEMM1 operand dtype

LAST_RESULT = {}        # test.py introspection (exec time etc.)


def _np_dt(name):
    return {"float32": np.float32, "bfloat16": ml_dtypes.bfloat16}[name]


# --------------------------------------------------------------------------
# CPU preprocessing
# --------------------------------------------------------------------------

def _preprocess(edge_index, n_nodes, blocks_per_core):
    """Sort edges (plus self-loops) by dst, pack into fixed-count edge tiles.

    Returns (srcs, segs, norms, T):
      srcs  [NCORES, 128, BPC*T] int32   src node id of edge p in tile g
      segs  [NCORES, 128, BPC*T] float32 dst % 128 (local row in block)
      norms [NCORES, 128, BPC*T] float32 dinv[src]*dinv[dst] (0 for padding)
      T = edge tiles per block (uniform across all cores/blocks)
    """
    nblk = NCORES * blocks_per_core
    src = np.asarray(edge_index[0], dtype=np.int64)
    dst = np.asarray(edge_index[1], dtype=np.int64)

    deg = np.bincount(dst, minlength=n_nodes).astype(np.float32) + 1.0
    dinv = (1.0 / np.sqrt(deg)).astype(np.float32)

    loops = np.arange(n_nodes, dtype=np.int64)
    all_src = np.concatenate([src, loops])
    all_dst = np.concatenate([dst, loops])
    norm = dinv[all_src] * dinv[all_dst]

    order = np.argsort(all_dst, kind="stable")
    s_src = all_src[order].astype(np.int32)
    s_dst = all_dst[order]
    s_norm = norm[order].astype(np.float32)

    blk = s_dst // P
    seg = (s_dst % P).astype(np.float32)
    counts = np.bincount(blk, minlength=nblk)
    T = max(1, int(math.ceil(counts.max() / P)))

    nt = blocks_per_core * T
    srcs = np.zeros((NCORES, P, nt), np.int32)
    segs = np.zeros((NCORES, P, nt), np.float32)
    norms = np.zeros((NCORES, P, nt), np.float32)

    starts = np.concatenate([[0], np.cumsum(counts)])
    for b in range(nblk):
        c, bl = divmod(b, blocks_per_core)
        lo, hi = int(starts[b]), int(starts[b + 1])
        n = hi - lo
        if n == 0:
            continue
        j = np.arange(n)
        g = bl * T + j // P
        p = j % P
        srcs[c, p, g] = s_src[lo:hi]
        segs[c, p, g] = seg[lo:hi]
        norms[c, p, g] = s_norm[lo:hi]
    return srcs, segs, norms, T


# --------------------------------------------------------------------------
# Device program
# --------------------------------------------------------------------------

def _build_program(f_in, hidden, ncls_pad, blocks_per_core, T, hdt_name,
                   h2dt_name, xdt_name):
    import concourse.bacc as bacc
    import concourse.bass as bass
    import concourse.mybir as mybir
    import concourse.tile as tile

    dt = mybir.dt
    name2dt = {"float32": dt.float32, "bfloat16": dt.bfloat16}
    hdt = name2dt[hdt_name]
    h2dt = name2dt[h2dt_name]
    xdt = name2dt[xdt_name]
    f32 = dt.float32

    shard = blocks_per_core * P
    n_pad = NCORES * shard
    nt = blocks_per_core * T
    kt = f_in // P  # k-tiles in GEMM1

    nc = bacc.Bacc(
        "TRN2",
        target_bir_lowering=False,
        debug=False,
        enable_asserts=False,
        num_devices=NCORES,
    )

    # Kernel I/O
    xt_d = nc.dram_tensor("xt", [f_in, shard], xdt, kind="ExternalInput")
    w1_d = nc.dram_tensor("w1", [P, kt * hidden], xdt, kind="ExternalInput")
    b1_d = nc.dram_tensor("b1", [P, 1], f32, kind="ExternalInput")
    w2_d = nc.dram_tensor("w2", [hidden, ncls_pad], f32, kind="ExternalInput")
    b2_d = nc.dram_tensor("b2t", [P, ncls_pad], f32, kind="ExternalInput")
    iota_d = nc.dram_tensor("iotaw", [P, T * P], f32, kind="ExternalInput")
    srcs_d = nc.dram_tensor("srcs", [P, nt], dt.int32, kind="ExternalInput")
    segs_d = nc.dram_tensor("segs", [P, nt], f32, kind="ExternalInput")
    norms_d = nc.dram_tensor("norms", [P, nt], f32, kind="ExternalInput")
    out_d = nc.dram_tensor("out", [shard, N_CLASSES], f32, kind="ExternalOutput")

    RG = [list(range(NCORES))]

    with tile.TileContext(nc) as tc:
        with (
            tc.tile_pool(name="const", bufs=1) as const,
            tc.tile_pool(name="dram", bufs=1, space="DRAM") as dram,
            tc.tile_pool(name="sb", bufs=3) as sb,
            tc.tile_pool(name="psum", bufs=2, space="PSUM") as psum,
        ):
            # Internal DRAM buffers
            h_ag_in = dram.tile([shard, hidden], hdt)
            h_full = dram.tile([n_pad, hidden], hdt, addr_space="Shared")
            h2_ag_in = dram.tile([shard, ncls_pad], h2dt)
            h2_full = dram.tile([n_pad, ncls_pad], h2dt, addr_space="Shared")

            # Constants into SBUF
            w1_sb = const.tile([P, kt * hidden], xdt)
            nc.sync.dma_start(out=w1_sb[:], in_=w1_d[:])
            b1_sb = const.tile([P, 1], f32)
            nc.sync.dma_start(out=b1_sb[:], in_=b1_d[:])
            w2_sb = const.tile([hidden, ncls_pad], f32)
            nc.sync.dma_start(out=w2_sb[:], in_=w2_d[:])
            b2_sb = const.tile([P, ncls_pad], f32)
            nc.sync.dma_start(out=b2_sb[:], in_=b2_d[:])
            iota_sb = const.tile([P, T * P], f32)
            nc.sync.dma_start(out=iota_sb[:], in_=iota_d[:])
            srcs_sb = const.tile([P, nt], dt.int32)
            nc.sync.dma_start(out=srcs_sb[:], in_=srcs_d[:])
            segs_sb = const.tile([P, nt], f32)
            nc.sync.dma_start(out=segs_sb[:], in_=segs_d[:])
            norms_sb = const.tile([P, nt], f32)
            nc.sync.dma_start(out=norms_sb[:], in_=norms_d[:])

            # ---------------- Phase 1: GEMM1 (h = x @ W1) ----------------
            for i in range(blocks_per_core):
                psum_h = psum.tile([P, hidden], f32, tag="psum_h")
                for k in range(kt):
                    xt_t = sb.tile([P, P], xdt, tag="xt", bufs=4)
                    nc.sync.dma_start(
                        out=xt_t[:],
                        in_=xt_d[k * P:(k + 1) * P, i * P:(i + 1) * P],
                    )
                    nc.tensor.matmul(
                        out=psum_h[:],
                        lhsT=xt_t[:],
                        rhs=w1_sb[:, k * hidden:(k + 1) * hidden],
                        start=(k == 0),
                        stop=(k == kt - 1),
                    )
                h_t = sb.tile([P, hidden], hdt, tag="h_t")
                nc.vector.tensor_copy(out=h_t[:], in_=psum_h[:])
                nc.sync.dma_start(
                    out=h_ag_in[i * P:(i + 1) * P, :], in_=h_t[:]
                )

            # ---------------- AllGather h ----------------
            nc.gpsimd.collective_compute(
                "AllGather",
                mybir.AluOpType.bypass,
                replica_groups=RG,
                ins=[h_ag_in[:]],
                outs=[h_full[:]],
            )

            # ---------------- Phase 2: Agg1 + relu + GEMM2 ----------------
            def build_selector(b, seldt):
                g0 = b * T
                sel = sb.tile([P, T * P], seldt, tag="sel")
                sel3 = sel[:].rearrange("p (t d) -> p t d", d=P)
                nc.vector.tensor_tensor(
                    out=sel3,
                    in0=iota_sb[:].rearrange("p (t d) -> p t d", d=P),
                    in1=segs_sb[:, g0:g0 + T].to_broadcast([P, T, P]),
                    op=mybir.AluOpType.is_equal,
                )
                nc.vector.tensor_tensor(
                    out=sel3,
                    in0=sel3,
                    in1=norms_sb[:, g0:g0 + T].to_broadcast([P, T, P]),
                    op=mybir.AluOpType.mult,
                )
                return sel

            for b in range(blocks_per_core):
                g0 = b * T
                msg = sb.tile([P, T * hidden], hdt, tag="msg")
                for t in range(T):
                    nc.gpsimd.indirect_dma_start(
                        out=msg[:, t * hidden:(t + 1) * hidden],
                        out_offset=None,
                        in_=h_full[:],
                        in_offset=bass.IndirectOffsetOnAxis(
                            ap=srcs_sb[:, g0 + t:g0 + t + 1], axis=0
                        ),
                    )
                sel = build_selector(b, hdt)
                psum1 = psum.tile([P, P], f32, tag="psum1")
                for t in range(T):
                    nc.tensor.matmul(
                        out=psum1[:],
                        lhsT=msg[:, t * hidden:(t + 1) * hidden],
                        rhs=sel[:, t * P:(t + 1) * P],
                        start=(t == 0),
                        stop=(t == T - 1),
                    )
                # psum1 = agg1^T : [hidden, dst]; relu(agg + b1) with b1 along
                # partitions.
                a1 = sb.tile([P, P], f32, tag="a1")
                nc.scalar.activation(
                    out=a1[:], in_=psum1[:],
                    func=mybir.ActivationFunctionType.Relu,
                    bias=b1_sb[:, 0:1],
                )
                psum2 = psum.tile([P, ncls_pad], f32, tag="psum2")
                nc.tensor.matmul(
                    out=psum2[:], lhsT=a1[:], rhs=w2_sb[:],
                    start=True, stop=True,
                )
                h2_t = sb.tile([P, ncls_pad], h2dt, tag="h2_t")
                nc.vector.tensor_copy(out=h2_t[:], in_=psum2[:])
                nc.sync.dma_start(
                    out=h2_ag_in[b * P:(b + 1) * P, :], in_=h2_t[:]
                )

            # ---------------- AllGather h2 ----------------
            nc.gpsimd.collective_compute(
                "AllGather",
                mybir.AluOpType.bypass,
                replica_groups=RG,
                ins=[h2_ag_in[:]],
                outs=[h2_full[:]],
            )

            # ---------------- Phase 3: Agg2 + bias + log_softmax ----------------
            for b in range(blocks_per_core):
                g0 = b * T
                msg2 = sb.tile([P, T * ncls_pad], h2dt, tag="msg2")
                for t in range(T):
                    nc.gpsimd.indirect_dma_start(
                        out=msg2[:, t * ncls_pad:(t + 1) * ncls_pad],
                        out_offset=None,
                        in_=h2_full[:],
                        in_offset=bass.IndirectOffsetOnAxis(
                            ap=srcs_sb[:, g0 + t:g0 + t + 1], axis=0
                        ),
                    )
                sel = build_selector(b, h2dt)
                psum_o = psum.tile([P, ncls_pad], f32, tag="psum_o")
                for t in range(T):
                    nc.tensor.matmul(
                        out=psum_o[:],
                        lhsT=sel[:, t * P:(t + 1) * P],
                        rhs=msg2[:, t * ncls_pad:(t + 1) * ncls_pad],
                        start=(t == 0),
                        stop=(t == T - 1),
                    )
                logits = sb.tile([P, N_CLASSES], f32, tag="logits")
                nc.vector.tensor_tensor(
                    out=logits[:], in0=psum_o[:, 0:N_CLASSES],
                    in1=b2_sb[:, 0:N_CLASSES], op=mybir.AluOpType.add,
                )
                negm = sb.tile([P, 1], f32, tag="negm")
                nc.vector.reduce_max(
                    out=negm[:], in_=logits[:], axis=mybir.AxisListType.X
                )
                nc.vector.tensor_scalar_mul(
                    out=negm[:], in0=negm[:], scalar1=-1.0
                )
                expv = sb.tile([P, N_CLASSES], f32, tag="expv")
                nc.scalar.activation(
                    out=expv[:], in_=logits[:],
                    func=mybir.ActivationFunctionType.Exp,
                    bias=negm[:, 0:1],
                )
                ssum = sb.tile([P, 1], f32, tag="ssum")
                nc.vector.reduce_sum(
                    out=ssum[:], in_=expv[:], axis=mybir.AxisListType.X
                )
                lns = sb.tile([P, 1], f32, tag="lns")
                nc.scalar.activation(
                    out=lns[:], in_=ssum[:],
                    func=mybir.ActivationFunctionType.Ln,
                )
                outt = sb.tile([P, N_CLASSES], f32, tag="outt")
                nc.vector.tensor_scalar(
                    out=outt[:], in0=logits[:],
                    scalar1=negm[:, 0:1], scalar2=lns[:, 0:1],
                    op0=mybir.AluOpType.add, op1=mybir.AluOpType.subtract,
                )
                nc.sync.dma_start(
                    out=out_d[b * P:(b + 1) * P, :], in_=outt[:]
                )

    nc.compile()
    return nc


# --------------------------------------------------------------------------
# Host orchestration
# --------------------------------------------------------------------------

def _run(x, edge_index, W1, b1, W2, b2, blocks_per_core):
    from concourse.bass_utils import run_bass_kernel_spmd

    global LAST_RESULT

    x = np.asarray(x, dtype=np.float32)
    W1 = np.asarray(W1, dtype=np.float32)
    b1v = np.asarray(b1, dtype=np.float32).reshape(-1)
    W2 = np.asarray(W2, dtype=np.float32)
    b2v = np.asarray(b2, dtype=np.float32).reshape(-1)

    n_nodes, f_in = x.shape
    hidden = W1.shape[1]
    ncls = W2.shape[1]
    ncls_pad = 64 if ncls <= 64 else int(math.ceil(ncls / P) * P)
    assert hidden == P and ncls == N_CLASSES

    shard = blocks_per_core * P
    n_pad = NCORES * shard
    assert n_pad >= n_nodes

    srcs, segs, norms, T = _preprocess(edge_index, n_nodes, blocks_per_core)

    nc = _build_program(
        f_in, hidden, ncls_pad, blocks_per_core, T,
        H_DTYPE, H2_DTYPE, X_DTYPE,
    )

    xdt_np = _np_dt(X_DTYPE)
    kt = f_in // P

    x_pad = np.zeros((n_pad, f_in), np.float32)
    x_pad[:n_nodes] = x
    w1r = np.ascontiguousarray(
        W1.reshape(kt, P, hidden).transpose(1, 0, 2).reshape(P, kt * hidden)
    ).astype(xdt_np)
    w2p = np.zeros((hidden, ncls_pad), np.float32)
    w2p[:, :ncls] = W2
    b2t = np.zeros((P, ncls_pad), np.float32)
    b2t[:, :ncls] = b2v[None, :]
    iotaw = np.ascontiguousarray(
        np.broadcast_to(
            np.tile(np.arange(P, dtype=np.float32), T), (P, T * P)
        )
    )

    in_maps = []
    for c in range(NCORES):
        xt_c = np.ascontiguousarray(
            x_pad[c * shard:(c + 1) * shard].T
        ).astype(xdt_np)
        in_maps.append({
            "xt": xt_c,
            "w1": w1r,
            "b1": b1v.reshape(P, 1).copy(),
            "w2": w2p,
            "b2t": b2t,
            "iotaw": iotaw,
            "srcs": np.ascontiguousarray(srcs[c]),
            "segs": np.ascontiguousarray(segs[c]),
            "norms": np.ascontiguousarray(norms[c]),
        })

    res = run_bass_kernel_spmd(
        nc, in_maps, core_ids=list(range(NCORES)),
        trace=TRACE, trace_kwargs=dict(TRACE_KWARGS),
    )
    LAST_RESULT = {
        "exec_time_ns": res.exec_time_ns,
        "mean_exec_time_ns": res.mean_exec_time_ns,
        "instructions_and_trace": res.instructions_and_trace,
        "profile_json": res.profile_json,
        "T": T,
        "nc": nc,
        "in_maps": in_maps,
    }
    out = np.concatenate([r["out"] for r in res.results], axis=0)
    return out[:n_nodes]


def kernel(x, edge_index, W1, b1, W2, b2):
    n_nodes = np.asarray(x).shape[0]
    blocks_per_core = int(math.ceil(n_nodes / (NCORES * P)))
    return _run(x, edge_index, W1, b1, W2, b2, blocks_per_core)



# revision 10
# speedup vs baseline: 1.5847x; 1.5847x over previous
"""GCN (2-layer, GCNConv + log_softmax) on 8 Trainium2 NeuronCores.

Strategy (1D node partition):
  - Nodes padded to N_PAD = 392*128, sharded contiguously: 49 blocks of 128
    dst-nodes per core.
  - CPU preprocessing: add self-loops, compute dinv=1/sqrt(deg), sort edges by
    dst, pack per (core, block) into edge tiles of 128.  dinv[src] is folded
    into the gathered feature tables (h, h2 stored pre-scaled), dinv[dst] is
    applied after each aggregation, so the selector is a pure one-hot
    (single is_equal on DVE; padding slots get seg=255).
  - Edge->feature gathers use the hardware gather instruction
    (gpsimd.dma_gather / InstDMAGatherAnt): amortizes SWDGE descriptor
    generation over a whole block of edges (the per-tile indirect-DMA variant
    paid ~1us fixed cost per 128 edges and dominated the runtime).  Indices
    are int16, so the node-row space is split at 32768: each block does a
    "lo" gather from rows [0, 32768) and a "hi" gather from a rebased view
    of rows [32768, n_pad); edges are packed into lo tiles then hi tiles.
  - On device per core:
      GEMM1: h = (x_shard @ W1) * dinv  (PE bf16, ACT-scaled copy)
      AllGather h -> full bf16 h table [n_pad, 128]
      Agg1 per dst block: lo+hi gathers, one-hot selector, segment-sum via PE
        matmul accumulation into PSUM [hid, dst]; * dinv[dst] (DVE), relu+b1
        (ACT), fused GEMM2 (bf16) -> h2 = (a1 @ W2) * dinv stored f32
        64-padded (gather rows must be a multiple of 256 bytes); store batched.
      AllGather h2 (f32, 64-wide)
      Agg2 per dst block: lo+hi gathers of h2[src], f32 selector, accumulate
        [dst, 64]; * dinv[dst] + b2 on the first 40 cols, log_softmax; store.
  - Host concatenates the 8 output shards and strips padding.
"""

import math

import numpy as np
import ml_dtypes

P = 128
NCORES = 8
HALF = 32768            # int16 gather index limit

# Full-problem constants (hardcoded per harness contract).
N_NODES = 50000
N_EDGES = 800000
F_IN = 512
HIDDEN = 128
N_CLASSES = 40
NCLS_PAD = 64           # h2 table width: 64 * 4B = 256B rows

# Runtime-tunable knobs (test.py may override before calling kernel()).
TRACE = False
TRACE_KWARGS = {}
CH = 1          # AllGather chunks (1 = single collective per layer)
SG = 7          # blocks per store DMA (must divide blocks-per-AG-chunk)

LAST_RESULT = {}        # test.py introspection (exec time etc.)


# --------------------------------------------------------------------------
# CPU preprocessing
# --------------------------------------------------------------------------

def _preprocess(edge_index, n_nodes, blocks_per_core, ch):
    """Sort edges (plus self-loops) by dst; pack per block into lo/hi tiles.

    Returns (idxlo, idxhi, segs, dinv_pad, T_lo, T_hi):
      idxlo [NCORES, 128, BPC*T_lo*8]  int16  gather indices, 16-wrapped
      idxhi [NCORES, 128, BPC*T_hi*8]  int16  (rebased by -32768)
      segs  [NCORES, 128, BPC*(T_lo+T_hi)] f32  dst%128 (255 for padding)
      dinv_pad [n_pad] f32
    """
    shard = blocks_per_core * P
    n_pad = NCORES * shard
    qsz = shard // ch
    nblk = NCORES * blocks_per_core
    src = np.asarray(edge_index[0], dtype=np.int64)
    dst = np.asarray(edge_index[1], dtype=np.int64)

    deg = np.bincount(dst, minlength=n_nodes).astype(np.float32) + 1.0
    dinv = (1.0 / np.sqrt(deg)).astype(np.float32)
    dinv_pad = np.zeros(n_pad, np.float32)
    dinv_pad[:n_nodes] = dinv

    loops = np.arange(n_nodes, dtype=np.int64)
    all_src = np.concatenate([src, loops])
    all_dst = np.concatenate([dst, loops])

    order = np.argsort(all_dst, kind="stable")
    s_src = all_src[order]
    s_dst = all_dst[order]

    # h_full row of node v when the AllGather is split into ch chunks:
    # chunk q of every core lands contiguously as [q][core][row-in-chunk].
    c_of = s_src // shard
    r0 = s_src % shard
    q_of = r0 // qsz
    s_row = ((q_of * NCORES + c_of) * qsz + r0 % qsz).astype(np.int64)

    blk = s_dst // P
    is_lo = s_row < HALF

    # per-block lo/hi counts -> uniform tile counts
    cnt_lo = np.bincount(blk[is_lo], minlength=nblk)
    cnt_hi = np.bincount(blk[~is_lo], minlength=nblk)
    T_lo = max(1, int(math.ceil(cnt_lo.max() / P)))
    T_hi = max(1, int(math.ceil(cnt_hi.max() / P)))
    T = T_lo + T_hi

    segs = np.full((NCORES, P, blocks_per_core * T), 255.0, np.float32)
    idxlo16 = np.zeros((NCORES, 16, blocks_per_core * T_lo * 8), np.int16)
    idxhi16 = np.zeros((NCORES, 16, blocks_per_core * T_hi * 8), np.int16)

    for b in range(nblk):
        c, bl = divmod(b, blocks_per_core)
        in_b = blk == b
        for lo_flag, idx_arr, T_part, t_base, w in (
            (True, idxlo16, T_lo, 0, T_lo * 8),
            (False, idxhi16, T_hi, T_lo, T_hi * 8),
        ):
            m = in_b & (is_lo == lo_flag)
            rows = s_row[m] - (0 if lo_flag else HALF)
            dloc = s_dst[m] % P
            s = np.arange(len(rows))
            # seg for slot s -> tile t_base + s//128, partition s%128
            segs[c, s % P, bl * T + t_base + s // P] = dloc
            # gather idx, 16-wrapped: slot s -> [s%16, s//16]
            idx_arr[c, s % 16, bl * w + s // 16] = rows.astype(np.int16)
    # replicate the 16-partition wrap across all 8 GPSIMD core stripes
    idxlo = np.tile(idxlo16, (1, 8, 1))
    idxhi = np.tile(idxhi16, (1, 8, 1))
    return idxlo, idxhi, segs, dinv_pad, T_lo, T_hi


# --------------------------------------------------------------------------
# Device program
# --------------------------------------------------------------------------

def _build_program(f_in, hidden, blocks_per_core, T_lo, T_hi, ch, sg):
    import concourse.bacc as bacc
    import concourse.mybir as mybir
    import concourse.tile as tile

    dt = mybir.dt
    bf16 = dt.bfloat16
    f32 = dt.float32

    T = T_lo + T_hi
    ncp = NCLS_PAD
    shard = blocks_per_core * P
    n_pad = NCORES * shard
    kt = f_in // P
    qb = blocks_per_core // ch       # blocks per AG chunk
    qsz = qb * P
    wlo, whi = T_lo * 8, T_hi * 8    # idx cols per block
    assert blocks_per_core % ch == 0 and qb % sg == 0

    nc = bacc.Bacc(
        "TRN2",
        target_bir_lowering=False,
        debug=False,
        enable_asserts=False,
        num_devices=NCORES,
    )

    xt_d = nc.dram_tensor("xt", [P, blocks_per_core * kt * P], bf16,
                          kind="ExternalInput")
    w1_d = nc.dram_tensor("w1", [P, kt * hidden], bf16, kind="ExternalInput")
    b1_d = nc.dram_tensor("b1", [P, 1], f32, kind="ExternalInput")
    w2_d = nc.dram_tensor("w2", [hidden, ncp], bf16, kind="ExternalInput")
    b2_d = nc.dram_tensor("b2t", [P, N_CLASSES], f32, kind="ExternalInput")
    iota_d = nc.dram_tensor("iotaw", [P, T * P], f32, kind="ExternalInput")
    ilo_d = nc.dram_tensor("idxlo", [P, blocks_per_core * wlo], dt.int16,
                           kind="ExternalInput")
    ihi_d = nc.dram_tensor("idxhi", [P, blocks_per_core * whi], dt.int16,
                           kind="ExternalInput")
    segs_d = nc.dram_tensor("segs", [P, blocks_per_core * T], f32,
                            kind="ExternalInput")
    dinvp_d = nc.dram_tensor("dinvp", [P, blocks_per_core], f32,
                             kind="ExternalInput")
    dinvf_d = nc.dram_tensor("dinvf", [P, shard], f32, kind="ExternalInput")
    out_d = nc.dram_tensor("out", [shard, N_CLASSES], f32,
                           kind="ExternalOutput")

    RG = [list(range(NCORES))]
    Relu = mybir.ActivationFunctionType.Relu
    Copy = mybir.ActivationFunctionType.Copy
    Exp = mybir.ActivationFunctionType.Exp
    Ln = mybir.ActivationFunctionType.Ln

    with tile.TileContext(nc) as tc:
        with (
            tc.tile_pool(name="const", bufs=1) as const,
            tc.tile_pool(name="dram", bufs=1, space="DRAM") as dram,
            tc.tile_pool(name="sb", bufs=3) as sb,
            tc.tile_pool(name="psum", bufs=2, space="PSUM") as psum,
        ):
            h_ag_in = dram.tile([shard, hidden], bf16)
            h_full = dram.tile([n_pad, hidden], bf16, addr_space="Shared")
            h2_ag_in = dram.tile([shard, ncp], f32)
            h2_full = dram.tile([n_pad, ncp], f32, addr_space="Shared")

            w1_sb = const.tile([P, kt * hidden], bf16)
            nc.sync.dma_start(out=w1_sb[:], in_=w1_d[:])
            b1_sb = const.tile([P, 1], f32)
            nc.sync.dma_start(out=b1_sb[:], in_=b1_d[:])
            w2_sb = const.tile([hidden, ncp], bf16)
            nc.sync.dma_start(out=w2_sb[:], in_=w2_d[:])
            b2_sb = const.tile([P, N_CLASSES], f32)
            nc.sync.dma_start(out=b2_sb[:], in_=b2_d[:])
            iota_sb = const.tile([P, T * P], f32)
            nc.sync.dma_start(out=iota_sb[:], in_=iota_d[:])
            ilo_sb = const.tile([P, blocks_per_core * wlo], dt.int16)
            nc.sync.dma_start(out=ilo_sb[:], in_=ilo_d[:])
            ihi_sb = const.tile([P, blocks_per_core * whi], dt.int16)
            nc.sync.dma_start(out=ihi_sb[:], in_=ihi_d[:])
            segs_sb = const.tile([P, blocks_per_core * T], f32)
            nc.sync.dma_start(out=segs_sb[:], in_=segs_d[:])
            dinvp_sb = const.tile([P, blocks_per_core], f32)
            nc.sync.dma_start(out=dinvp_sb[:], in_=dinvp_d[:])
            dinvf_sb = const.tile([P, shard], f32)
            nc.sync.dma_start(out=dinvf_sb[:], in_=dinvf_d[:])
            xt_sb = const.tile([P, blocks_per_core * kt * P], bf16)
            for q in range(ch):
                w = qb * kt * P
                nc.sync.dma_start(
                    out=xt_sb[:, q * w:(q + 1) * w],
                    in_=xt_d[:, q * w:(q + 1) * w],
                )

            iota3 = iota_sb[:].rearrange("p (t d) -> p t d", d=P)

            def build_selector(b, seldt, tag):
                g0 = b * T
                sel = sb.tile([P, T * P], seldt, tag=tag)
                nc.vector.tensor_tensor(
                    out=sel[:].rearrange("p (t d) -> p t d", d=P),
                    in0=iota3,
                    in1=segs_sb[:, g0:g0 + T].to_broadcast([P, T, P]),
                    op=mybir.AluOpType.is_equal,
                )
                return sel

            def gather_block(b, table, F, dtype, tag):
                msg = sb.tile([P, T * F], dtype, tag=tag)
                nc.gpsimd.dma_gather(
                    out_ap=msg[:, :T_lo * F].rearrange(
                        "p (t f) -> p t f", f=F),
                    in_ap=table[0:HALF, :],
                    idxs_ap=ilo_sb[:, b * wlo:(b + 1) * wlo],
                    num_idxs=T_lo * P,
                    num_idxs_reg=T_lo * P,
                    elem_size=F,
                    single_packet=False,
                    queue_num=0,
                )
                nc.gpsimd.dma_gather(
                    out_ap=msg[:, T_lo * F:].rearrange(
                        "p (t f) -> p t f", f=F),
                    in_ap=table[HALF:, :],
                    idxs_ap=ihi_sb[:, b * whi:(b + 1) * whi],
                    num_idxs=T_hi * P,
                    num_idxs_reg=T_hi * P,
                    elem_size=F,
                    single_packet=False,
                    queue_num=0,
                )
                return msg

            # ---------------- Phase 1: GEMM1 (h = (x @ W1) * dinv) --------
            for q in range(ch):
                h_acc = None
                for j in range(qb):
                    i = q * qb + j
                    ps = psum.tile([P, P], f32, tag="psA")
                    for k in range(kt):
                        c0 = (i * kt + k) * P
                        nc.tensor.matmul(
                            out=ps[:],
                            lhsT=xt_sb[:, c0:c0 + P],
                            rhs=w1_sb[:, k * hidden:(k + 1) * hidden],
                            start=(k == 0),
                            stop=(k == kt - 1),
                        )
                    jj = j % sg
                    if jj == 0:
                        h_acc = sb.tile([P, sg * hidden], bf16, tag="h_acc",
                                        bufs=2)
                    nc.scalar.activation(
                        out=h_acc[:, jj * hidden:(jj + 1) * hidden],
                        in_=ps[:], func=Copy, scale=dinvp_sb[:, i:i + 1],
                    )
                    if jj == sg - 1:
                        b0 = i - sg + 1
                        nc.sync.dma_start(
                            out=h_ag_in[b0 * P:(i + 1) * P, :].rearrange(
                                "(g p) h -> p g h", p=P),
                            in_=h_acc[:].rearrange("p (g h) -> p g h",
                                                   h=hidden),
                        )
                nc.gpsimd.collective_compute(
                    "AllGather",
                    mybir.AluOpType.bypass,
                    replica_groups=RG,
                    ins=[h_ag_in[q * qsz:(q + 1) * qsz, :]],
                    outs=[h_full[q * qsz * NCORES:(q + 1) * qsz * NCORES, :]],
                )

            # ------- Phase 2: Agg1 * dinv, relu + b1, GEMM2, * dinv -------
            for q in range(ch):
                h2_acc = None
                for j in range(qb):
                    b = q * qb + j
                    msg = gather_block(b, h_full, hidden, bf16, "msg")
                    sel = build_selector(b, bf16, "sel2")
                    ps1 = psum.tile([P, P], f32, tag="psA")
                    for t in range(T):
                        nc.tensor.matmul(
                            out=ps1[:],
                            lhsT=msg[:, t * hidden:(t + 1) * hidden],
                            rhs=sel[:, t * P:(t + 1) * P],
                            start=(t == 0),
                            stop=(t == T - 1),
                        )
                    # ps1 = agg1^T : [hid, dst]; * dinv[dst] then relu(.+b1)
                    t1 = sb.tile([P, P], f32, tag="t1")
                    nc.vector.tensor_tensor(
                        out=t1[:], in0=ps1[:],
                        in1=dinvf_sb[:, b * P:(b + 1) * P],
                        op=mybir.AluOpType.mult,
                    )
                    a1 = sb.tile([P, P], bf16, tag="a1")
                    nc.scalar.activation(
                        out=a1[:], in_=t1[:], func=Relu, bias=b1_sb[:, 0:1],
                    )
                    ps2 = psum.tile([P, ncp], f32, tag="psB")
                    nc.tensor.matmul(
                        out=ps2[:], lhsT=a1[:], rhs=w2_sb[:],
                        start=True, stop=True,
                    )
                    jj = j % sg
                    if jj == 0:
                        h2_acc = sb.tile([P, sg * ncp], f32, tag="h2_acc",
                                         bufs=2)
                    nc.scalar.activation(
                        out=h2_acc[:, jj * ncp:(jj + 1) * ncp],
                        in_=ps2[:], func=Copy, scale=dinvp_sb[:, b:b + 1],
                    )
                    if jj == sg - 1:
                        b0 = b - sg + 1
                        nc.sync.dma_start(
                            out=h2_ag_in[b0 * P:(b + 1) * P, :].rearrange(
                                "(g p) c -> p g c", p=P),
                            in_=h2_acc[:].rearrange("p (g c) -> p g c",
                                                    c=ncp),
                        )
                nc.gpsimd.collective_compute(
                    "AllGather",
                    mybir.AluOpType.bypass,
                    replica_groups=RG,
                    ins=[h2_ag_in[q * qsz:(q + 1) * qsz, :]],
                    outs=[h2_full[q * qsz * NCORES:(q + 1) * qsz * NCORES, :]],
                )

            # -------- Phase 3: Agg2 * dinv + b2, log_softmax --------------
            out_acc = None
            for b in range(blocks_per_core):
                msg2 = gather_block(b, h2_full, ncp, f32, "msg2")
                sel = build_selector(b, f32, "sel3")
                ps_o = psum.tile([P, ncp], f32, tag="psB")
                for t in range(T):
                    nc.tensor.matmul(
                        out=ps_o[:],
                        lhsT=sel[:, t * P:(t + 1) * P],
                        rhs=msg2[:, t * ncp:(t + 1) * ncp],
                        start=(t == 0),
                        stop=(t == T - 1),
                    )
                u = sb.tile([P, N_CLASSES], f32, tag="u")
                nc.scalar.activation(
                    out=u[:], in_=ps_o[:, 0:N_CLASSES], func=Copy,
                    scale=dinvp_sb[:, b:b + 1],
                )
                logits = sb.tile([P, N_CLASSES], f32, tag="logits")
                nc.vector.tensor_tensor(
                    out=logits[:], in0=u[:], in1=b2_sb[:],
                    op=mybir.AluOpType.add,
                )
                negm = sb.tile([P, 1], f32, tag="negm")
                nc.vector.reduce_max(
                    out=negm[:], in_=logits[:], axis=mybir.AxisListType.X
                )
                nc.vector.tensor_scalar_mul(
                    out=negm[:], in0=negm[:], scalar1=-1.0
                )
                expv = sb.tile([P, N_CLASSES], f32, tag="expv")
                nc.scalar.activation(
                    out=expv[:], in_=logits[:], func=Exp, bias=negm[:, 0:1],
                )
                ssum = sb.tile([P, 1], f32, tag="ssum")
                nc.vector.reduce_sum(
                    out=ssum[:], in_=expv[:], axis=mybir.AxisListType.X
                )
                lns = sb.tile([P, 1], f32, tag="lns")
                nc.scalar.activation(out=lns[:], in_=ssum[:], func=Ln)
                jj = b % sg
                if jj == 0:
                    out_acc = sb.tile([P, sg * N_CLASSES], f32, tag="out_acc",
                                      bufs=2)
                nc.vector.tensor_scalar(
                    out=out_acc[:, jj * N_CLASSES:(jj + 1) * N_CLASSES],
                    in0=logits[:],
                    scalar1=negm[:, 0:1], scalar2=lns[:, 0:1],
                    op0=mybir.AluOpType.add, op1=mybir.AluOpType.subtract,
                )
                if jj == sg - 1:
                    b0 = b - sg + 1
                    nc.sync.dma_start(
                        out=out_d[b0 * P:(b + 1) * P, :].rearrange(
                            "(g p) c -> p g c", p=P),
                        in_=out_acc[:].rearrange("p (g c) -> p g c",
                                                 c=N_CLASSES),
                    )

    nc.compile()
    return nc


# --------------------------------------------------------------------------
# Host orchestration
# --------------------------------------------------------------------------

def _run(x, edge_index, W1, b1, W2, b2, blocks_per_core):
    from concourse.bass_utils import run_bass_kernel_spmd

    global LAST_RESULT

    x = np.asarray(x, dtype=np.float32)
    W1 = np.asarray(W1, dtype=np.float32)
    b1v = np.asarray(b1, dtype=np.float32).reshape(-1)
    W2 = np.asarray(W2, dtype=np.float32)
    b2v = np.asarray(b2, dtype=np.float32).reshape(-1)

    n_nodes, f_in = x.shape
    hidden = W1.shape[1]
    ncls = W2.shape[1]
    assert hidden == P and ncls == N_CLASSES

    shard = blocks_per_core * P
    n_pad = NCORES * shard

    idxlo, idxhi, segs, dinv_pad, T_lo, T_hi = _preprocess(
        edge_index, n_nodes, blocks_per_core, CH
    )
    T = T_lo + T_hi

    nc = _build_program(f_in, hidden, blocks_per_core, T_lo, T_hi, CH, SG)

    kt = f_in // P
    bf = ml_dtypes.bfloat16

    x_pad = np.zeros((n_pad, f_in), np.float32)
    x_pad[:n_nodes] = x
    w1r = np.ascontiguousarray(
        W1.reshape(kt, P, hidden).transpose(1, 0, 2).reshape(P, kt * hidden)
    ).astype(bf)
    w2p = np.zeros((hidden, NCLS_PAD), np.float32)
    w2p[:, :ncls] = W2
    b2t = np.ascontiguousarray(
        np.broadcast_to(b2v[None, :], (P, ncls))
    ).astype(np.float32)
    iotaw = np.ascontiguousarray(
        np.broadcast_to(
            np.tile(np.arange(P, dtype=np.float32), T), (P, T * P)
        )
    )

    in_maps = []
    for c in range(NCORES):
        xs = x_pad[c * shard:(c + 1) * shard]
        xt4 = np.ascontiguousarray(
            xs.reshape(blocks_per_core, P, kt, P).transpose(3, 0, 2, 1)
            .reshape(P, blocks_per_core * kt * P)
        ).astype(bf)
        dshard = dinv_pad[c * shard:(c + 1) * shard]
        dinvp = np.ascontiguousarray(
            dshard.reshape(blocks_per_core, P).T
        ).astype(np.float32)
        dinvf = np.ascontiguousarray(
            np.broadcast_to(dshard[None, :], (P, shard))
        ).astype(np.float32)
        in_maps.append({
            "xt": xt4,
            "w1": w1r,
            "b1": b1v.reshape(P, 1).copy(),
            "w2": w2p.astype(bf),
            "b2t": b2t,
            "iotaw": iotaw,
            "idxlo": np.ascontiguousarray(idxlo[c]),
            "idxhi": np.ascontiguousarray(idxhi[c]),
            "segs": np.ascontiguousarray(segs[c]),
            "dinvp": dinvp,
            "dinvf": dinvf,
        })

    res = run_bass_kernel_spmd(
        nc, in_maps, core_ids=list(range(NCORES)),
        trace=TRACE, trace_kwargs=dict(TRACE_KWARGS),
    )
    LAST_RESULT = {
        "exec_time_ns": res.exec_time_ns,
        "mean_exec_time_ns": res.mean_exec_time_ns,
        "instructions_and_trace": res.instructions_and_trace,
        "profile_json": res.profile_json,
        "T": T,
        "nc": nc,
        "in_maps": in_maps,
    }
    out = np.concatenate([r["out"] for r in res.results], axis=0)
    return out[:n_nodes]


def kernel(x, edge_index, W1, b1, W2, b2):
    n_nodes = np.asarray(x).shape[0]
    blocks_per_core = int(math.ceil(n_nodes / (NCORES * P)))
    return _run(x, edge_index, W1, b1, W2, b2, blocks_per_core)
